# revision 6
# baseline (speedup 1.0000x reference)
"""Trainium2 Bass kernel for nn_Blur2: depthwise 4x4 blur (upfirdn2d-style,
pad=(2,1), unit stride) over input [8, 128, 256, 256] f32.

Strategy: pure data parallel over the 1024 independent (n, c) planes --
128 planes per NeuronCore. The blur kernel is rank-1 (uh x uw, entries
0.25/0.75 -- exact in fp16), which enables a two-pass PE scheme that
costs ~516 PE cycles per plane-tier instead of the ~1020 of the fused
4-shift banded matmul:

  pass 1 (H-conv, transposed output): stationary = the image tile X
  itself ([x-rows, w-chunk] slices), moving = a static banded H matrix
  AH [128 x-rows, 128 out-rows]; PSUM gets y^T[w, m] (two 128-wide w
  chunks side by side). Clipped bands in AH encode the H zero-padding.

  stage: y^T is cast PSUM f32 -> SBUF fp16 (DVE/scalar alternating).

  pass 2 (W-conv): stationary = the staged y^T chunk ([w, m]), moving =
  small static banded W matrices AW0/AW1 [128, 130]; PSUM gets
  out[m, w'] in normal orientation. The band crossing at w'=127..129
  is handled by a 4-column accumulating matmul.

Engine balance per tier of 8 planes: 3 planes stay on the fused path
(keeps PE ~97% of the DMA-paced oct time; fused planes need only ONE
PSUM->SBUF pass) and 5 planes take the two-pass scheme. DVE+scalar
land at ~85% with stages/out-copies alternating between them. Output
DMA triggers ride the otherwise-idle GpSimd ring so the scalar engine
only does copies. The HAM clock governor halves the core clock after
~3.4us of low activity, so junk matmuls pad PE activity through the
output-drain/epilogue tail, and the first oct's load is column-split
so the real PE stream starts as soon as the first planes land.

Precision: single fp16 cast of the input; blur weights are exact in
fp16; fp16 y^T staging adds < 1e-3 rel err (tolerance is 2e-2).

DMA: 8 planes per DRAM row (4 KB fp16); per oct the two H-tier row
blocks are host-interleaved into ONE [128, 8KB] load; output goes out
as per-tier [128, 4KB] stores (halves the trailing-store drain).
Loads ride the sync HWDGE ring, stores the gpsimd ring.
"""
import sys

for _p in ("/opt/trn_rl_repo", "/opt/pypackages"):
    if _p not in sys.path:
        sys.path.insert(0, _p)

import contextlib

import numpy as np


def _install_ntff_hook_shim():
    """The agent image's antenv lacks axon_hooks, which bass_utils needs
    for trace=True under axon. Provide it in sys.modules, backed by
    trn_agent_boot's ctypes NTFF shim."""
    import types

    if "antenv.axon_hooks" in sys.modules:
        return
    mod = types.ModuleType("antenv.axon_hooks")
    state = {"hook": None, "tried": False}

    def set_axon_ntff_profile_hook(hook):
        state["hook"] = hook

    def get_axon_ntff_profile_hook():
        if state["hook"] is None and not state["tried"]:
            state["tried"] = True
            try:
                from trn_agent_boot.trn_boot import _ntff_profile_via_ctypes

                state["hook"] = _ntff_profile_via_ctypes("/opt/axon/libaxon_pjrt.so")
            except Exception:
                state["hook"] = None
        return state["hook"]

    mod.set_axon_ntff_profile_hook = set_axon_ntff_profile_hook
    mod.get_axon_ntff_profile_hook = get_axon_ntff_profile_hook
    sys.modules["antenv.axon_hooks"] = mod
    try:
        import antenv

        antenv.axon_hooks = mod
    except ImportError:
        pass


_install_ntff_hook_shim()

import concourse.bacc as bacc
import concourse.tile as tile
from concourse import mybir
from concourse.bass_utils import run_bass_kernel_spmd

N_CORES = 8
H = W = 256
PLANES = 1024 // N_CORES  # 128 per core
O = 8  # planes packed per SBUF/DRAM row (4KB fp16)
NOCT = PLANES // O  # 16 oct-groups per core
QP = 4  # planes per PSUM tile in the non-separable fallback path
NFS = 2  # fused planes per tier in the separable scheme
OW = O * W

# M-tile layout along H per plane:
#   tier A: out rows [0, 127)   from x rows [0, 128)
#   tier B: out rows [127, 252) from x rows [125, 253)
#   remainder: out rows [252, 256) from x rows [250, 256), stacked across
#   groups of RG=16 octs (96 partitions, 4 out rows per plane-slot)
MA, MB = 127, 125
RG = 16

# per W-shift i: out cols [wl, wh), reading x cols [cl, ch)  (tap = w-2+i)
SHIFT_RANGES = {
    0: (2, 256, 0, 254),
    1: (1, 256, 0, 255),
    2: (0, 256, 0, 256),
    3: (0, 255, 1, 256),
}
SHIFT_ORDER = [2, 0, 1, 3]  # full-range shift first so start=True covers all


def _separable(wk: np.ndarray):
    """Rank-1 factorization wk = outer(uh, uw); returns (uh, uw) or None."""
    u, s, vt = np.linalg.svd(wk.astype(np.float64))
    if s[1] > 1e-6 * s[0]:
        return None
    uh = u[:, 0] * np.sqrt(s[0])
    uw = vt[0] * np.sqrt(s[0])
    if uh.sum() < 0:
        uh, uw = -uh, -uw
    return uh, uw


def _make_weights(wk: np.ndarray):
    """wk: flipped 4x4 kernel. Packed fp16 weights, one 128-col matrix per
    W-shift (cols padded with zeros past MA/MB so NumWeights==128 enables
    the PE Fast-Weight-Load path): wa/wb [128, 4*128], wr [96, 4*64]
    (block-diag 16x(6->4))."""
    wa = np.zeros((128, 4, 128), np.float32)
    for k in range(128):
        for m in range(MA):
            d = k - m + 2
            if 0 <= d <= 3:
                wa[k, :, m] = wk[d, :]
    wb = np.zeros((128, 4, 128), np.float32)
    for k in range(128):
        for m in range(MB):
            d = k - m
            if 0 <= d <= 3:
                wb[k, :, m] = wk[d, :]
    wr = np.zeros((RG * 6, 4, RG * 4), np.float32)
    for b in range(RG):
        for r in range(6):
            for c in range(4):
                d = r - c
                if 0 <= d <= 3:
                    wr[6 * b + r, :, 4 * b + c] = wk[d, :]
    return (
        wa.reshape(128, 4 * 128).astype(np.float16),
        wb.reshape(128, 4 * 128).astype(np.float16),
        wr.reshape(RG * 6, 4 * RG * 4).astype(np.float16),
    )


def _make_sep_weights(uh: np.ndarray, uw: np.ndarray):
    """Banded matrices for the two-pass separable scheme.
    aha/ahb [128, 128]: pass-1 moving operand, aha[k, m] = uh[k-m+2]
    (tier A, H-pad clipped; cols m>=MA zero), ahb[k, m] = uh[k-m].
    aw0/aw1 [128, 132]: pass-2 moving operands; aw0 covers out cols
    w' in [0, 130) from y cols w in [0, 128); aw1 covers w' in
    [126, 256) from w in [128, 256) (first 4 cols are the band
    crossing, accumulated onto aw0's output)."""
    aha = np.zeros((128, 128), np.float32)
    ahb = np.zeros((128, 128), np.float32)
    for k in range(128):
        for m in range(MA):
            d = k - m + 2
            if 0 <= d <= 3:
                aha[k, m] = uh[d]
        for m in range(MB):
            d = k - m
            if 0 <= d <= 3:
                ahb[k, m] = uh[d]
    aw0 = np.zeros((128, 132), np.float32)
    aw1 = np.zeros((128, 132), np.float32)
    for w in range(128):
        for j in range(130):
            d = w - j + 2  # w' = j
            if 0 <= d <= 3:
                aw0[w, j] = uw[d]
            d = w - j + 4  # w' = 126 + j (global w = 128 + w)
            if 0 <= d <= 3:
                aw1[w, j] = uw[d]
    return (
        aha.astype(np.float16),
        ahb.astype(np.float16),
        aw0.astype(np.float16),
        aw1.astype(np.float16),
    )


def _build_program(noct: int = NOCT, sep: bool = True):
    nc = bacc.Bacc("TRN2", target_bir_lowering=False, debug=False)
    f16, f32 = mybir.dt.float16, mybir.dt.float32

    # xs: per oct, row r holds tier-A x row r (cols 0:O*W) interleaved with
    # tier-B x row 125+r (cols O*W:2*O*W) -> one 8KB-per-partition load.
    d_xs = nc.dram_tensor("xs", [noct, 128, 2 * OW], f16, kind="ExternalInput").ap()
    d_xr = nc.dram_tensor("xr", [noct, 6, OW], f16, kind="ExternalInput").ap()
    d_wa = nc.dram_tensor("wa", [128, 4 * 128], f16, kind="ExternalInput").ap()
    d_wb = nc.dram_tensor("wb", [128, 4 * 128], f16, kind="ExternalInput").ap()
    d_wr = nc.dram_tensor("wr", [RG * 6, 4 * RG * 4], f16, kind="ExternalInput").ap()
    if sep:
        d_aha = nc.dram_tensor("aha", [128, 128], f16, kind="ExternalInput").ap()
        d_ahb = nc.dram_tensor("ahb", [128, 128], f16, kind="ExternalInput").ap()
        d_aw0 = nc.dram_tensor("aw0", [128, 132], f16, kind="ExternalInput").ap()
        d_aw1 = nc.dram_tensor("aw1", [128, 132], f16, kind="ExternalInput").ap()
    # out: per oct, row r holds out rows r (tier A) and 127+r (tier B);
    # junk at (127, A) and (125..128, B). Remainder rows in d_or.
    d_out = nc.dram_tensor("out", [noct, 128, 2 * OW], f16, kind="ExternalOutput").ap()
    d_or = nc.dram_tensor("outr", [noct, 4, OW], f16, kind="ExternalOutput").ap()

    rem_groups = [(s, min(RG, noct - s)) for s in range(0, noct, RG)]

    with tile.TileContext(nc) as tc, contextlib.ExitStack() as ctx:
        wpool = ctx.enter_context(tc.tile_pool(name="wpool", bufs=1))
        xin = ctx.enter_context(tc.tile_pool(name="xin", bufs=4))
        xinr = ctx.enter_context(tc.tile_pool(name="xinr", bufs=1))
        psum = ctx.enter_context(tc.tile_pool(name="psum", bufs=2, space="PSUM"))
        psr = ctx.enter_context(tc.tile_pool(name="psr", bufs=1, space="PSUM"))
        outp = ctx.enter_context(tc.tile_pool(name="outp", bufs=6))
        outr = ctx.enter_context(tc.tile_pool(name="outr", bufs=2))
        if sep:
            psyp = ctx.enter_context(tc.tile_pool(name="psyp", bufs=2, space="PSUM"))
            psop = ctx.enter_context(tc.tile_pool(name="psop", bufs=2, space="PSUM"))
            ysp = ctx.enter_context(tc.tile_pool(name="ysp", bufs=3))

        # PE warmup: junk matmuls with no data dependencies, issued before
        # any real work. They run during the DMA/program-upload ramp and
        # lift the HAM clock gate to 2.4 GHz before the real stream starts.
        warm = wpool.tile([128, 2 * W], f16, tag="warm")
        nc.vector.memset(warm[:], 0.0)
        psW = psr.tile([128, 2 * W], f32, tag="psR")
        for _ in range(24):
            nc.tensor.matmul(
                psW[:, :], warm[:, :128], warm[:, :],
                start=True, stop=True, skip_group_check=True,
            )

        t_wa = wpool.tile([128, 4 * 128], f16, tag="wa")
        nc.scalar.dma_start(out=t_wa[:], in_=d_wa)
        t_wb = wpool.tile([128, 4 * 128], f16, tag="wb")
        nc.scalar.dma_start(out=t_wb[:], in_=d_wb)
        t_wr = wpool.tile([RG * 6, 4 * RG * 4], f16, tag="wr")
        nc.scalar.dma_start(out=t_wr[:], in_=d_wr)
        if sep:
            t_aha = wpool.tile([128, 128], f16, tag="aha")
            nc.scalar.dma_start(out=t_aha[:], in_=d_aha)
            t_ahb = wpool.tile([128, 128], f16, tag="ahb")
            nc.scalar.dma_start(out=t_ahb[:], in_=d_ahb)
            t_aw0 = wpool.tile([128, 132], f16, tag="aw0")
            nc.scalar.dma_start(out=t_aw0[:], in_=d_aw0)
            t_aw1 = wpool.tile([128, 132], f16, tag="aw1")
            nc.scalar.dma_start(out=t_aw1[:], in_=d_aw1)

        def conv_mms(ps, wt, xt, xrows, qbase, npl=QP):
            """4 shifts x npl planes accumulating matmuls into the psum tile
            ps [128, npl*W] (per-bank first mm gets start=True). Plane p of
            the merged input tile xt sits at cols [p*W, (p+1)*W)."""
            last = (SHIFT_ORDER[-1], npl - 1)
            for i in SHIFT_ORDER:
                wl, wh, cl, ch = SHIFT_RANGES[i]
                lhsT = wt[:xrows, i * 128 : i * 128 + 128]
                if i == 2:
                    # full-range shift goes first; each PSUM BANK must be
                    # armed by exactly ONE start=True matmul covering all
                    # planes in it (start=True resets the bank's touched
                    # map, so a second start in the same bank would turn
                    # the earlier plane's accumulations into overwrites).
                    qq = 0
                    while qq < npl:
                        step = 2 if qq + 1 < npl else 1
                        nc.tensor.matmul(
                            ps[:128, qq * W : (qq + step) * W],
                            lhsT,
                            xt[:xrows, (qbase + qq) * W : (qbase + qq + step) * W],
                            start=True,
                            stop=False,
                            skip_group_check=True,
                        )
                        qq += step
                    continue
                for q in range(npl):
                    nc.tensor.matmul(
                        ps[:128, q * W + wl : q * W + wh],
                        lhsT,
                        xt[:xrows, (qbase + q) * W + cl : (qbase + q) * W + ch],
                        start=False,
                        stop=((i, q) == last),
                        skip_group_check=True,
                    )

        cp = [0]

        def copy_alt(dst, src):
            """PSUM -> SBUF fp16 copy alternating scalar/vector."""
            if cp[0] % 2 == 0:
                nc.scalar.copy(dst, src)
            else:
                nc.vector.tensor_copy(dst, src)
            cp[0] += 1

        def pass1(psy, ah, xt, toff, planes):
            """H-conv with X as stationary: psy[:, (2i+c)*128:(2i+c+1)*128]
            = y^T[w-chunk c, m] for plane planes[i]."""
            n = len(planes)
            for i, p in enumerate(planes):
                for c in range(2):
                    nc.tensor.matmul(
                        psy[:128, (2 * i + c) * 128 : (2 * i + c + 1) * 128],
                        xt[:128, toff + p * W + c * 128 : toff + p * W + c * 128 + 128],
                        ah[:128, :],
                        start=True,
                        stop=(i == n - 1 and c == 1),
                        skip_group_check=True,
                    )

        def pass2(pso, ys, planes):
            """W-conv with staged y^T as stationary: pso[:, i*W:(i+1)*W] =
            out[m, w'] for plane planes[i]."""
            n = len(planes)
            for i in range(n):
                c0 = ys[:128, (2 * i) * 128 : (2 * i) * 128 + 128]
                c1 = ys[:128, (2 * i + 1) * 128 : (2 * i + 1) * 128 + 128]
                lo = i * W
                # start=True arms the bank and touches [lo, lo+130); the
                # second matmul (start=False) then ACCUMULATES on the
                # touched band-crossing cols [lo+126, lo+130) and
                # first-touch-OVERWRITES [lo+130, lo+256) -- one matmul
                # covers both behaviors under the PSUM touched-map.
                nc.tensor.matmul(
                    pso[:128, lo : lo + 130], c0, t_aw0[:128, 0:130],
                    start=True, stop=False, skip_group_check=True,
                )
                nc.tensor.matmul(
                    pso[:128, lo + 126 : lo + 256], c1, t_aw1[:128, 0:130],
                    start=False, stop=(i == n - 1), skip_group_check=True,
                )

        T_GROUPS = [(2, 3), (4, 5), (6, 7)]  # transpose-path plane groups

        ri = 0
        for g in range(noct):
            tab = xin.tile([128, 2 * OW], f16, tag="tab")
            if g == 0:
                # column-split the very first load so the PE stream can
                # start on the first planes as soon as they land
                nc.sync.dma_start(out=tab[:, 0 : NFS * W], in_=d_xs[g, :, 0 : NFS * W])
                nc.sync.dma_start(out=tab[:, NFS * W : OW], in_=d_xs[g, :, NFS * W : OW])
                nc.sync.dma_start(
                    out=tab[:, OW : OW + NFS * W], in_=d_xs[g, :, OW : OW + NFS * W]
                )
                nc.sync.dma_start(
                    out=tab[:, OW + NFS * W :], in_=d_xs[g, :, OW + NFS * W :]
                )
            else:
                nc.sync.dma_start(out=tab[:], in_=d_xs[g])

            oab = outp.tile([128, 2 * OW], f16, tag="oab")
            for tier in range(2):
                base = tier * O
                toff = tier * OW
                if sep:
                    wt = t_wa if tier == 0 else t_wb
                    ah = t_aha if tier == 0 else t_ahb
                    # fused planes 0..NFS-1
                    psF = psum.tile([128, NFS * W], f32, tag="psA")
                    conv_mms(psF, wt, tab, 128, base, NFS)
                    # pass 1 for all transpose groups, stages right behind
                    ys_tiles = []
                    for grp in T_GROUPS:
                        psy = psyp.tile([128, len(grp) * 256], f32, tag="psY")
                        pass1(psy, ah, tab, toff, grp)
                        ys = ysp.tile([128, len(grp) * 256], f16, tag="ys")
                        copy_alt(ys[:, :], psy[:, :])
                        ys_tiles.append(ys)
                    copy_alt(oab[:, base * W : (base + NFS) * W], psF[:, :])
                    # pass 2 + out-copy per group
                    for grp, ys in zip(T_GROUPS, ys_tiles):
                        pso = psop.tile([128, len(grp) * W], f32, tag="psO")
                        pass2(pso, ys, grp)
                        p0 = grp[0]
                        copy_alt(
                            oab[:, (base + p0) * W : (base + p0 + len(grp)) * W],
                            pso[:, :],
                        )
                else:
                    for h in range(O // QP):
                        ps = psum.tile([128, QP * W], f32, tag="psA")
                        conv_mms(ps, t_wa if tier == 0 else t_wb, tab, 128, base + h * QP)
                        hw = QP * W // 2
                        lo = (base + h * QP) * W
                        copy_alt(oab[:, lo : lo + hw], ps[:, :hw])
                        copy_alt(oab[:, lo + hw : lo + 2 * hw], ps[:, hw:])
                # per-tier store on the gpsimd ring: SWDGE descriptor
                # generation (~1us/store) is compute-paced anyway, and it
                # keeps DMA triggers off the copy engines and the sync
                # load queue.
                nc.gpsimd.dma_start(
                    out=d_out[g, :, toff : toff + OW], in_=oab[:, toff : toff + OW]
                )


            # stacked remainder: input rows come straight from DRAM, so
            # emit early to keep them off the kernel tail
            if ri < len(rem_groups) and g == min(2 * (ri + 1), noct - 1):
                s, gsz = rem_groups[ri]
                ri += 1
                tr = xinr.tile([RG * 6, OW], f16, tag="tr")
                nc.sync.dma_start(out=tr[: 6 * gsz, :], in_=d_xr[s : s + gsz])
                orr = outr.tile([RG * 4, OW], f16, tag="orr")
                for h in range(O // QP):
                    psR = psr.tile([RG * 4, QP * W], f32, tag="psR")
                    last = (SHIFT_ORDER[-1], QP - 1)
                    for i in SHIFT_ORDER:
                        wl, wh, cl, ch = SHIFT_RANGES[i]
                        lhsT = t_wr[: 6 * gsz, i * RG * 4 : i * RG * 4 + 4 * gsz]
                        for q in range(QP):
                            nc.tensor.matmul(
                                psR[: 4 * gsz, q * W + wl : q * W + wh],
                                lhsT,
                                tr[: 6 * gsz, (h * QP + q) * W + cl : (h * QP + q) * W + ch],
                                start=(i == SHIFT_ORDER[0] and q % 2 == 0),
                                stop=((i, q) == last),
                                skip_group_check=True,
                            )
                    copy_alt(
                        orr[: 4 * gsz, h * QP * W : (h + 1) * QP * W],
                        psR[: 4 * gsz, :],
                    )
                nc.gpsimd.dma_start(out=d_or[s : s + gsz], in_=orr[: 4 * gsz])

        # tail PE activity: junk matmuls that run while the last output
        # stores drain, keeping the HAM clock governor at full rate
        # through the drain + semaphore-reset epilogue.
        psT = psr.tile([128, 2 * W], f32, tag="psR")
        for _ in range(12):
            nc.tensor.matmul(
                psT[:, :], warm[:, :128], warm[:, :],
                start=True, stop=True, skip_group_check=True,
            )

    nc.compile()
    return nc


_CACHE = {}


def _get_program(noct: int = NOCT, sep: bool = True):
    key = (noct, sep)
    if key not in _CACHE:
        _CACHE[key] = _build_program(noct, sep)
    return _CACHE[key]


def _run(x: np.ndarray, wk: np.ndarray, trace: bool = False):
    """x: [P, 256, 256] f32 full stack of planes (P divisible by 8*O),
    wk: flipped 4x4 kernel. Returns ([P, 256, 256] f32, exec_time_ns|None)."""
    P = x.shape[0]
    oper = P // (N_CORES * O)
    hi = x.astype(np.float16)
    # oct-pack: [P/O, O, H, W] -> [P/O, H, O, W] -> [P/O, H, O*W]
    xso = (
        hi.reshape(P // O, O, H, W)
        .transpose(0, 2, 1, 3)
        .reshape(P // O, H, OW)
    )
    # interleave tier-A rows 0:128 with tier-B rows 125:253 -> 8KB DMA rows
    xs3 = np.stack([xso[:, 0:128], xso[:, 125:253]], axis=2).reshape(
        P // O, 128, 2 * OW
    )
    xrem = np.ascontiguousarray(xso[:, 250:256])  # [P/O, 6, O*W]

    wa, wb, wr = _make_weights(wk)
    sep = _separable(wk)
    nc = _get_program(oper, sep is not None)

    in_maps = []
    for c in range(N_CORES):
        m = {
            "xs": np.ascontiguousarray(xs3[c * oper : (c + 1) * oper]),
            "xr": xrem[c * oper : (c + 1) * oper],
            "wa": wa,
            "wb": wb,
            "wr": wr,
        }
        if sep is not None:
            aha, ahb, aw0, aw1 = _make_sep_weights(*sep)
            m.update({"aha": aha, "ahb": ahb, "aw0": aw0, "aw1": aw1})
        in_maps.append(m)
    res = run_bass_kernel_spmd(nc, in_maps, list(range(N_CORES)), trace=trace)
    outq = np.concatenate([r["out"] for r in res.results], axis=0)
    outq = outq.reshape(P // O, 128, 2, OW)
    outrem = np.concatenate([r["outr"] for r in res.results], axis=0)  # [P/O,4,O*W]
    full = np.concatenate(
        [outq[:, 0:127, 0], outq[:, 0:125, 1], outrem], axis=1
    )  # [P/O, 256, O*W]
    out = (
        full.reshape(P // O, H, O, W)
        .transpose(0, 2, 1, 3)
        .reshape(P, H, W)
        .astype(np.float32)
    )
    return np.ascontiguousarray(out), res.exec_time_ns


def kernel(input: np.ndarray, kernel: np.ndarray) -> np.ndarray:
    x = np.asarray(input, dtype=np.float32)
    k = np.asarray(kernel, dtype=np.float32)
    n, c, h, w = x.shape
    wk = np.flip(k, (0, 1)).copy()  # correlation weights
    out, _ = _run(x.reshape(n * c, h, w), wk, trace=False)
    return out.reshape(n, c, h, w)


# revision 7
# speedup vs baseline: 1.1555x; 1.1555x over previous
"""Trainium2 Bass kernel for nn_Blur2: depthwise 4x4 blur (upfirdn2d-style,
pad=(2,1), unit stride) over input [8, 128, 256, 256] f32.

Strategy: pure data parallel over the 1024 independent (n, c) planes --
128 planes per NeuronCore. The blur kernel is rank-1 (uh x uw, entries
0.25/0.75 -- exact in fp16), which enables a two-pass PE scheme that
costs ~516 PE cycles per plane-tier instead of the ~1020 of the fused
4-shift banded matmul:

  pass 1 (H-conv, transposed output): stationary = the image tile X
  itself ([x-rows, w-chunk] slices), moving = a static banded H matrix
  AH [128 x-rows, 128 out-rows]; PSUM gets y^T[w, m] (two 128-wide w
  chunks side by side). Clipped bands in AH encode the H zero-padding.

  stage: y^T is cast PSUM f32 -> SBUF fp16 (DVE/scalar alternating).

  pass 2 (W-conv): stationary = the staged y^T chunk ([w, m]), moving =
  small static banded W matrices AW0/AW1 [128, 130]; PSUM gets
  out[m, w'] in normal orientation. The band crossing at w'=127..129
  is handled by a 4-column accumulating matmul.

Engine balance per tier of 8 planes: 3 planes stay on the fused path
(keeps PE ~97% of the DMA-paced oct time; fused planes need only ONE
PSUM->SBUF pass) and 5 planes take the two-pass scheme. DVE+scalar
land at ~85% with stages/out-copies alternating between them. Output
DMA triggers ride the otherwise-idle GpSimd ring so the scalar engine
only does copies. The HAM clock governor halves the core clock after
~3.4us of low activity, so junk matmuls pad PE activity through the
output-drain/epilogue tail, and the first oct's load is column-split
so the real PE stream starts as soon as the first planes land.

Precision: single fp16 cast of the input; blur weights are exact in
fp16; fp16 y^T staging adds < 1e-3 rel err (tolerance is 2e-2).

DMA: 8 planes per DRAM row (4 KB fp16); per oct the two H-tier row
blocks are host-interleaved into ONE [128, 8KB] load; output goes out
as per-tier [128, 4KB] stores (halves the trailing-store drain).
Loads ride the sync HWDGE ring, stores the gpsimd ring.
"""
import sys

for _p in ("/opt/trn_rl_repo", "/opt/pypackages"):
    if _p not in sys.path:
        sys.path.insert(0, _p)

import contextlib

import numpy as np


def _install_ntff_hook_shim():
    """The agent image's antenv lacks axon_hooks, which bass_utils needs
    for trace=True under axon. Provide it in sys.modules, backed by
    trn_agent_boot's ctypes NTFF shim."""
    import types

    if "antenv.axon_hooks" in sys.modules:
        return
    mod = types.ModuleType("antenv.axon_hooks")
    state = {"hook": None, "tried": False}

    def set_axon_ntff_profile_hook(hook):
        state["hook"] = hook

    def get_axon_ntff_profile_hook():
        if state["hook"] is None and not state["tried"]:
            state["tried"] = True
            try:
                from trn_agent_boot.trn_boot import _ntff_profile_via_ctypes

                state["hook"] = _ntff_profile_via_ctypes("/opt/axon/libaxon_pjrt.so")
            except Exception:
                state["hook"] = None
        return state["hook"]

    mod.set_axon_ntff_profile_hook = set_axon_ntff_profile_hook
    mod.get_axon_ntff_profile_hook = get_axon_ntff_profile_hook
    sys.modules["antenv.axon_hooks"] = mod
    try:
        import antenv

        antenv.axon_hooks = mod
    except ImportError:
        pass


_install_ntff_hook_shim()

import concourse.bacc as bacc
import concourse.tile as tile
from concourse import mybir
from concourse.bass_utils import run_bass_kernel_spmd

N_CORES = 8
H = W = 256
PLANES = 1024 // N_CORES  # 128 per core
O = 8  # planes packed per SBUF/DRAM row (4KB fp16)
NOCT = PLANES // O  # 16 oct-groups per core
QP = 4  # planes per PSUM tile in the non-separable fallback path
NFS = 3  # fused planes per tier in the separable scheme
OW = O * W

# M-tile layout along H per plane:
#   tier A: out rows [0, 127)   from x rows [0, 128)
#   tier B: out rows [127, 252) from x rows [125, 253)
#   remainder: out rows [252, 256) from x rows [250, 256), stacked across
#   groups of RG=16 octs (96 partitions, 4 out rows per plane-slot)
MA, MB = 127, 125
RG = 16

# per W-shift i: out cols [wl, wh), reading x cols [cl, ch)  (tap = w-2+i)
SHIFT_RANGES = {
    0: (2, 256, 0, 254),
    1: (1, 256, 0, 255),
    2: (0, 256, 0, 256),
    3: (0, 255, 1, 256),
}
SHIFT_ORDER = [2, 0, 1, 3]  # full-range shift first so start=True covers all


def _separable(wk: np.ndarray):
    """Rank-1 factorization wk = outer(uh, uw); returns (uh, uw) or None."""
    u, s, vt = np.linalg.svd(wk.astype(np.float64))
    if s[1] > 1e-6 * s[0]:
        return None
    uh = u[:, 0] * np.sqrt(s[0])
    uw = vt[0] * np.sqrt(s[0])
    if uh.sum() < 0:
        uh, uw = -uh, -uw
    return uh, uw


def _make_weights(wk: np.ndarray):
    """wk: flipped 4x4 kernel. Packed fp16 weights, one 128-col matrix per
    W-shift (cols padded with zeros past MA/MB so NumWeights==128 enables
    the PE Fast-Weight-Load path): wa/wb [128, 4*128], wr [96, 4*64]
    (block-diag 16x(6->4))."""
    wa = np.zeros((128, 4, 128), np.float32)
    for k in range(128):
        for m in range(MA):
            d = k - m + 2
            if 0 <= d <= 3:
                wa[k, :, m] = wk[d, :]
    wb = np.zeros((128, 4, 128), np.float32)
    for k in range(128):
        for m in range(MB):
            d = k - m
            if 0 <= d <= 3:
                wb[k, :, m] = wk[d, :]
    wr = np.zeros((RG * 6, 4, RG * 4), np.float32)
    for b in range(RG):
        for r in range(6):
            for c in range(4):
                d = r - c
                if 0 <= d <= 3:
                    wr[6 * b + r, :, 4 * b + c] = wk[d, :]
    return (
        wa.reshape(128, 4 * 128).astype(np.float16),
        wb.reshape(128, 4 * 128).astype(np.float16),
        wr.reshape(RG * 6, 4 * RG * 4).astype(np.float16),
    )


def _make_sep_weights(uh: np.ndarray, uw: np.ndarray):
    """Banded matrices for the two-pass separable scheme.
    aha/ahb [128, 128]: pass-1 moving operand, aha[k, m] = uh[k-m+2]
    (tier A, H-pad clipped; cols m>=MA zero), ahb[k, m] = uh[k-m].
    aw0/aw1 [128, 132]: pass-2 moving operands; aw0 covers out cols
    w' in [0, 130) from y cols w in [0, 128); aw1 covers w' in
    [126, 256) from w in [128, 256) (first 4 cols are the band
    crossing, accumulated onto aw0's output)."""
    aha = np.zeros((128, 128), np.float32)
    ahb = np.zeros((128, 128), np.float32)
    for k in range(128):
        for m in range(MA):
            d = k - m + 2
            if 0 <= d <= 3:
                aha[k, m] = uh[d]
        for m in range(MB):
            d = k - m
            if 0 <= d <= 3:
                ahb[k, m] = uh[d]
    aw0 = np.zeros((128, 132), np.float32)
    aw1 = np.zeros((128, 132), np.float32)
    for w in range(128):
        for j in range(130):
            d = w - j + 2  # w' = j
            if 0 <= d <= 3:
                aw0[w, j] = uw[d]
            d = w - j + 4  # w' = 126 + j (global w = 128 + w)
            if 0 <= d <= 3:
                aw1[w, j] = uw[d]
    return (
        aha.astype(np.float16),
        ahb.astype(np.float16),
        aw0.astype(np.float16),
        aw1.astype(np.float16),
    )


def _build_program(noct: int = NOCT, sep: bool = True):
    nc = bacc.Bacc("TRN2", target_bir_lowering=False, debug=False)
    f16, f32 = mybir.dt.float16, mybir.dt.float32

    # xs: per oct, row r holds tier-A x row r (cols 0:O*W) interleaved with
    # tier-B x row 125+r (cols O*W:2*O*W) -> one 8KB-per-partition load.
    d_xs = nc.dram_tensor("xs", [noct, 128, 2 * OW], f16, kind="ExternalInput").ap()
    d_xr = nc.dram_tensor("xr", [noct, 6, OW], f16, kind="ExternalInput").ap()
    d_wa = nc.dram_tensor("wa", [128, 4 * 128], f16, kind="ExternalInput").ap()
    d_wb = nc.dram_tensor("wb", [128, 4 * 128], f16, kind="ExternalInput").ap()
    d_wr = nc.dram_tensor("wr", [RG * 6, 4 * RG * 4], f16, kind="ExternalInput").ap()
    if sep:
        d_aha = nc.dram_tensor("aha", [128, 128], f16, kind="ExternalInput").ap()
        d_ahb = nc.dram_tensor("ahb", [128, 128], f16, kind="ExternalInput").ap()
        d_aw0 = nc.dram_tensor("aw0", [128, 132], f16, kind="ExternalInput").ap()
        d_aw1 = nc.dram_tensor("aw1", [128, 132], f16, kind="ExternalInput").ap()
    # out: per oct, row r holds out rows r (tier A) and 127+r (tier B);
    # junk at (127, A) and (125..128, B). Remainder rows in d_or.
    d_out = nc.dram_tensor("out", [noct, 128, 2 * OW], f16, kind="ExternalOutput").ap()
    d_or = nc.dram_tensor("outr", [noct, 4, OW], f16, kind="ExternalOutput").ap()

    rem_groups = [(s, min(RG, noct - s)) for s in range(0, noct, RG)]

    with tile.TileContext(nc) as tc, contextlib.ExitStack() as ctx:
        wpool = ctx.enter_context(tc.tile_pool(name="wpool", bufs=1))
        xin = ctx.enter_context(tc.tile_pool(name="xin", bufs=4))
        xinr = ctx.enter_context(tc.tile_pool(name="xinr", bufs=1))
        psum = ctx.enter_context(tc.tile_pool(name="psum", bufs=2, space="PSUM"))
        outp = ctx.enter_context(tc.tile_pool(name="outp", bufs=4))
        outr = ctx.enter_context(tc.tile_pool(name="outr", bufs=2))
        if sep:
            psyp = ctx.enter_context(tc.tile_pool(name="psyp", bufs=2, space="PSUM"))
            psop = ctx.enter_context(tc.tile_pool(name="psop", bufs=2, space="PSUM"))
            ysp = ctx.enter_context(tc.tile_pool(name="ysp", bufs=3))

        # PE warmup: junk matmuls with no data dependencies, issued before
        # any real work. They run during the DMA/program-upload ramp and
        # lift the HAM clock gate to 2.4 GHz before the real stream starts.
        warm = wpool.tile([128, 2 * W], f16, tag="warm")
        nc.vector.memset(warm[:], 0.0)
        psW = psum.tile([128, 2 * W], f32, tag="psA")
        for _ in range(24):
            nc.tensor.matmul(
                psW[:, :], warm[:, :128], warm[:, :],
                start=True, stop=True, skip_group_check=True,
            )

        t_wa = wpool.tile([128, 4 * 128], f16, tag="wa")
        nc.scalar.dma_start(out=t_wa[:], in_=d_wa)
        t_wb = wpool.tile([128, 4 * 128], f16, tag="wb")
        nc.scalar.dma_start(out=t_wb[:], in_=d_wb)
        t_wr = wpool.tile([RG * 6, 4 * RG * 4], f16, tag="wr")
        nc.scalar.dma_start(out=t_wr[:], in_=d_wr)
        if sep:
            t_aha = wpool.tile([128, 128], f16, tag="aha")
            nc.scalar.dma_start(out=t_aha[:], in_=d_aha)
            t_ahb = wpool.tile([128, 128], f16, tag="ahb")
            nc.scalar.dma_start(out=t_ahb[:], in_=d_ahb)
            t_aw0 = wpool.tile([128, 132], f16, tag="aw0")
            nc.scalar.dma_start(out=t_aw0[:], in_=d_aw0)
            t_aw1 = wpool.tile([128, 132], f16, tag="aw1")
            nc.scalar.dma_start(out=t_aw1[:], in_=d_aw1)

        def conv_mms(ps, wt, xt, xrows, qbase, npl=QP):
            """4 shifts x npl planes accumulating matmuls into the psum tile
            ps [128, npl*W] (per-bank first mm gets start=True). Plane p of
            the merged input tile xt sits at cols [p*W, (p+1)*W)."""
            last = (SHIFT_ORDER[-1], npl - 1)
            for i in SHIFT_ORDER:
                wl, wh, cl, ch = SHIFT_RANGES[i]
                lhsT = wt[:xrows, i * 128 : i * 128 + 128]
                if i == 2:
                    # full-range shift goes first; each PSUM BANK must be
                    # armed by exactly ONE start=True matmul covering all
                    # planes in it (start=True resets the bank's touched
                    # map, so a second start in the same bank would turn
                    # the earlier plane's accumulations into overwrites).
                    qq = 0
                    while qq < npl:
                        step = 2 if qq + 1 < npl else 1
                        nc.tensor.matmul(
                            ps[:128, qq * W : (qq + step) * W],
                            lhsT,
                            xt[:xrows, (qbase + qq) * W : (qbase + qq + step) * W],
                            start=True,
                            stop=False,
                            skip_group_check=True,
                        )
                        qq += step
                    continue
                for q in range(npl):
                    nc.tensor.matmul(
                        ps[:128, q * W + wl : q * W + wh],
                        lhsT,
                        xt[:xrows, (qbase + q) * W + cl : (qbase + q) * W + ch],
                        start=False,
                        stop=((i, q) == last),
                        skip_group_check=True,
                    )

        cp = [0]

        def copy_alt(dst, src):
            """PSUM -> SBUF fp16 copy alternating scalar/vector."""
            if cp[0] % 2 == 0:
                nc.scalar.copy(dst, src)
            else:
                nc.vector.tensor_copy(dst, src)
            cp[0] += 1

        def pass1(psy, ah, xt, toff, planes):
            """H-conv with X as stationary: psy[:, (2i+c)*128:(2i+c+1)*128]
            = y^T[w-chunk c, m] for plane planes[i]."""
            n = len(planes)
            for i, p in enumerate(planes):
                for c in range(2):
                    nc.tensor.matmul(
                        psy[:128, (2 * i + c) * 128 : (2 * i + c + 1) * 128],
                        xt[:128, toff + p * W + c * 128 : toff + p * W + c * 128 + 128],
                        ah[:128, :],
                        start=True,
                        stop=(i == n - 1 and c == 1),
                        skip_group_check=True,
                    )

        def pass2(pso, ys, planes):
            """W-conv with staged y^T as stationary: pso[:, i*W:(i+1)*W] =
            out[m, w'] for plane planes[i]."""
            n = len(planes)
            for i in range(n):
                c0 = ys[:128, (2 * i) * 128 : (2 * i) * 128 + 128]
                c1 = ys[:128, (2 * i + 1) * 128 : (2 * i + 1) * 128 + 128]
                lo = i * W
                # start=True arms the bank and touches [lo, lo+130); the
                # second matmul (start=False) then ACCUMULATES on the
                # touched band-crossing cols [lo+126, lo+130) and
                # first-touch-OVERWRITES [lo+130, lo+256) -- one matmul
                # covers both behaviors under the PSUM touched-map.
                nc.tensor.matmul(
                    pso[:128, lo : lo + 130], c0, t_aw0[:128, 0:130],
                    start=True, stop=False, skip_group_check=True,
                )
                nc.tensor.matmul(
                    pso[:128, lo + 126 : lo + 256], c1, t_aw1[:128, 0:130],
                    start=False, stop=(i == n - 1), skip_group_check=True,
                )

        T_GROUPS = [(3, 4), (5, 6), (7,)]  # transpose-path plane groups

        ri = 0
        for g in range(noct):
            tab = xin.tile([128, 2 * OW], f16, tag="tab")
            if g == 0:
                # column-split the very first load so the PE stream can
                # start on the first planes as soon as they land
                nc.sync.dma_start(out=tab[:, 0 : NFS * W], in_=d_xs[g, :, 0 : NFS * W])
                nc.sync.dma_start(out=tab[:, NFS * W : OW], in_=d_xs[g, :, NFS * W : OW])
                nc.sync.dma_start(
                    out=tab[:, OW : OW + NFS * W], in_=d_xs[g, :, OW : OW + NFS * W]
                )
                nc.sync.dma_start(
                    out=tab[:, OW + NFS * W :], in_=d_xs[g, :, OW + NFS * W :]
                )
            else:
                nc.sync.dma_start(out=tab[:], in_=d_xs[g])

            oab = outp.tile([128, 2 * OW], f16, tag="oab")
            for tier in range(2):
                base = tier * O
                toff = tier * OW
                if sep:
                    wt = t_wa if tier == 0 else t_wb
                    ah = t_aha if tier == 0 else t_ahb
                    # fused planes 0..NFS-1
                    psF = psum.tile([128, NFS * W], f32, tag="psA")
                    conv_mms(psF, wt, tab, 128, base, NFS)
                    # pass 1 for all transpose groups, stages right behind
                    ys_tiles = []
                    for grp in T_GROUPS:
                        psy = psyp.tile([128, len(grp) * 256], f32, tag="psY")
                        pass1(psy, ah, tab, toff, grp)
                        ys = ysp.tile([128, len(grp) * 256], f16, tag="ys")
                        copy_alt(ys[:, :], psy[:, :])
                        ys_tiles.append(ys)
                    copy_alt(oab[:, base * W : (base + NFS) * W], psF[:, :])
                    # pass 2 + out-copy per group
                    for grp, ys in zip(T_GROUPS, ys_tiles):
                        pso = psop.tile([128, len(grp) * W], f32, tag="psO")
                        pass2(pso, ys, grp)
                        p0 = grp[0]
                        copy_alt(
                            oab[:, (base + p0) * W : (base + p0 + len(grp)) * W],
                            pso[:, :],
                        )
                else:
                    for h in range(O // QP):
                        ps = psum.tile([128, QP * W], f32, tag="psA")
                        conv_mms(ps, t_wa if tier == 0 else t_wb, tab, 128, base + h * QP)
                        hw = QP * W // 2
                        lo = (base + h * QP) * W
                        copy_alt(oab[:, lo : lo + hw], ps[:, :hw])
                        copy_alt(oab[:, lo + hw : lo + 2 * hw], ps[:, hw:])
                # per-tier store on the gpsimd ring: SWDGE descriptor
                # generation (~1us/store) is compute-paced anyway, and it
                # keeps DMA triggers off the copy engines and the sync
                # load queue.
                nc.gpsimd.dma_start(
                    out=d_out[g, :, toff : toff + OW], in_=oab[:, toff : toff + OW]
                )


            # stacked remainder: input rows come straight from DRAM, so
            # emit early to keep them off the kernel tail
            if ri < len(rem_groups) and g == min(2 * (ri + 1), noct - 1):
                s, gsz = rem_groups[ri]
                ri += 1
                tr = xinr.tile([RG * 6, OW], f16, tag="tr")
                nc.sync.dma_start(out=tr[: 6 * gsz, :], in_=d_xr[s : s + gsz])
                orr = outr.tile([RG * 4, OW], f16, tag="orr")
                for h in range(O // QP):
                    psR = psum.tile([RG * 4, QP * W], f32, tag="psA")
                    last = (SHIFT_ORDER[-1], QP - 1)
                    for i in SHIFT_ORDER:
                        wl, wh, cl, ch = SHIFT_RANGES[i]
                        lhsT = t_wr[: 6 * gsz, i * RG * 4 : i * RG * 4 + 4 * gsz]
                        for q in range(QP):
                            nc.tensor.matmul(
                                psR[: 4 * gsz, q * W + wl : q * W + wh],
                                lhsT,
                                tr[: 6 * gsz, (h * QP + q) * W + cl : (h * QP + q) * W + ch],
                                start=(i == SHIFT_ORDER[0] and q % 2 == 0),
                                stop=((i, q) == last),
                                skip_group_check=True,
                            )
                    copy_alt(
                        orr[: 4 * gsz, h * QP * W : (h + 1) * QP * W],
                        psR[: 4 * gsz, :],
                    )
                nc.gpsimd.dma_start(out=d_or[s : s + gsz], in_=orr[: 4 * gsz])

        # tail PE activity: junk matmuls that run while the last output
        # stores drain, keeping the HAM clock governor at full rate
        # through the drain + semaphore-reset epilogue.
        psT = psum.tile([128, 2 * W], f32, tag="psA")
        for _ in range(12):
            nc.tensor.matmul(
                psT[:, :], warm[:, :128], warm[:, :],
                start=True, stop=True, skip_group_check=True,
            )

    nc.compile()
    return nc


_CACHE = {}


def _get_program(noct: int = NOCT, sep: bool = True):
    key = (noct, sep)
    if key not in _CACHE:
        _CACHE[key] = _build_program(noct, sep)
    return _CACHE[key]


def _run(x: np.ndarray, wk: np.ndarray, trace: bool = False):
    """x: [P, 256, 256] f32 full stack of planes (P divisible by 8*O),
    wk: flipped 4x4 kernel. Returns ([P, 256, 256] f32, exec_time_ns|None)."""
    P = x.shape[0]
    oper = P // (N_CORES * O)
    hi = x.astype(np.float16)
    # oct-pack: [P/O, O, H, W] -> [P/O, H, O, W] -> [P/O, H, O*W]
    xso = (
        hi.reshape(P // O, O, H, W)
        .transpose(0, 2, 1, 3)
        .reshape(P // O, H, OW)
    )
    # interleave tier-A rows 0:128 with tier-B rows 125:253 -> 8KB DMA rows
    xs3 = np.stack([xso[:, 0:128], xso[:, 125:253]], axis=2).reshape(
        P // O, 128, 2 * OW
    )
    xrem = np.ascontiguousarray(xso[:, 250:256])  # [P/O, 6, O*W]

    wa, wb, wr = _make_weights(wk)
    sep = _separable(wk)
    nc = _get_program(oper, sep is not None)

    in_maps = []
    for c in range(N_CORES):
        m = {
            "xs": np.ascontiguousarray(xs3[c * oper : (c + 1) * oper]),
            "xr": xrem[c * oper : (c + 1) * oper],
            "wa": wa,
            "wb": wb,
            "wr": wr,
        }
        if sep is not None:
            aha, ahb, aw0, aw1 = _make_sep_weights(*sep)
            m.update({"aha": aha, "ahb": ahb, "aw0": aw0, "aw1": aw1})
        in_maps.append(m)
    res = run_bass_kernel_spmd(nc, in_maps, list(range(N_CORES)), trace=trace)
    outq = np.concatenate([r["out"] for r in res.results], axis=0)
    outq = outq.reshape(P // O, 128, 2, OW)
    outrem = np.concatenate([r["outr"] for r in res.results], axis=0)  # [P/O,4,O*W]
    full = np.concatenate(
        [outq[:, 0:127, 0], outq[:, 0:125, 1], outrem], axis=1
    )  # [P/O, 256, O*W]
    out = (
        full.reshape(P // O, H, O, W)
        .transpose(0, 2, 1, 3)
        .reshape(P, H, W)
        .astype(np.float32)
    )
    return np.ascontiguousarray(out), res.exec_time_ns


def kernel(input: np.ndarray, kernel: np.ndarray) -> np.ndarray:
    x = np.asarray(input, dtype=np.float32)
    k = np.asarray(kernel, dtype=np.float32)
    n, c, h, w = x.shape
    wk = np.flip(k, (0, 1)).copy()  # correlation weights
    out, _ = _run(x.reshape(n * c, h, w), wk, trace=False)
    return out.reshape(n, c, h, w)
